# revision 9
# baseline (speedup 1.0000x reference)
"""InteractionNetworkLayer on 8 TRN2 cores.

Sharding: core = (b, q); b = batch (2), q = node-quarter (4). Each core owns
nodes [q*4096,(q+1)*4096) of batch b, and (via host argsort by receiver) the
contiguous run of edges whose receiver falls in that range. Edges are grouped
by 128-node windows (32 groups/core), each group padded to T tiles of 128
edges (T = global max, data-dependent).

Device dataflow (feature-major activations, bf16 matmuls):
  - send feats: indirect-DMA gather of bf16 node rows + DMA-transpose
  - recv feats: never gathered; folded through Y = nodes_window @ W1_recv and
    a one-hot matmul (receivers sorted => window-local)
  - edge MLP: h1T[h,e] accumulated in PSUM from (bias, Y@onehot, W1@sndT,
    W1@esT); relu on ACT; enewT = eW2@hT
  - aggregation: aggT[De,n] += enew_em.T-matmul with edge-major one-hots
  - node MLP: feature-major end-to-end, residual via host-transposed nodes
"""
import numpy as np
import ml_dtypes
from concourse import bacc, mybir
import concourse.bass as bass
from concourse.tile import TileContext
from concourse.masks import make_identity
from concourse.bass_utils import run_bass_kernel_spmd

BF16 = mybir.dt.bfloat16
F32 = mybir.dt.float32
I32 = mybir.dt.int32
P = 128
B, N, E, Dn, De, H = 2, 16384, 131072, 256, 128, 256
Q = 4              # cores per batch
NPC = N // Q       # 4096 nodes per core
G = NPC // P       # 32 groups per core
bf = ml_dtypes.bfloat16

_cache = {}


def _build(T):
    if T in _cache:
        return _cache[T]
    TT = G * T
    Ep = TT * P
    nc = bacc.Bacc(None, target_bir_lowering=False)

    d_nb = nc.declare_dram_parameter("nodes_bf", [N, Dn], BF16, isOutput=False)
    d_nbo = nc.declare_dram_parameter("nodes_bf_own", [NPC, Dn], BF16, isOutput=False)
    d_nto = nc.declare_dram_parameter("nodesT_own", [Dn, NPC], F32, isOutput=False)
    d_es = nc.declare_dram_parameter("edges_s", [Ep, De], F32, isOutput=False)
    d_ew1 = nc.declare_dram_parameter("ew1", [P, 5 * 256], BF16, isOutput=False)
    d_ew2 = nc.declare_dram_parameter("ew2", [P, 2 * 128], BF16, isOutput=False)
    d_nw1 = nc.declare_dram_parameter("nw1", [P, 3 * 256], BF16, isOutput=False)
    d_nw2 = nc.declare_dram_parameter("nw2", [P, 2 * 256], BF16, isOutput=False)
    d_eb1 = nc.declare_dram_parameter("eb1", [1, 256], BF16, isOutput=False)
    d_eb2 = nc.declare_dram_parameter("eb2", [1, 128], BF16, isOutput=False)
    d_nb1 = nc.declare_dram_parameter("nb1", [1, 256], BF16, isOutput=False)
    d_nb2 = nc.declare_dram_parameter("nb2", [1, 256], BF16, isOutput=False)
    d_sn = nc.declare_dram_parameter("g_snd", [P, TT], I32, isOutput=False)
    d_rc = nc.declare_dram_parameter("g_rc", [P, TT], F32, isOutput=False)
    d_rr = nc.declare_dram_parameter("g_rr", [1, Ep], BF16, isOutput=False)
    d_eo = nc.declare_dram_parameter("eout", [Ep, De], F32, isOutput=True)
    d_no = nc.declare_dram_parameter("noutT", [Dn, NPC], F32, isOutput=True)

    with TileContext(nc) as tc:
        with (
            tc.tile_pool(name="const", bufs=1) as cp,
            tc.tile_pool(name="wts", bufs=1) as wp,
            tc.tile_pool(name="work", bufs=4) as sp,
            tc.tile_pool(name="grp", bufs=3) as gp,
            tc.tile_pool(name="ph1", bufs=3, space="PSUM") as ph1,
            tc.tile_pool(name="psml", bufs=1, space="PSUM") as psml,
            tc.tile_pool(name="pagg", bufs=1, space="PSUM") as pagg,
            tc.tile_pool(name="pmisc", bufs=1, space="PSUM") as pmisc,
        ):
            # constants
            ii = cp.tile([P, P], I32)
            nc.gpsimd.iota(ii[:], pattern=[[1, P]], base=0, channel_multiplier=0)
            iota_row = cp.tile([P, P], F32)
            nc.vector.tensor_copy(out=iota_row[:], in_=ii[:])
            ic = cp.tile([P, 1], I32)
            nc.gpsimd.iota(ic[:], pattern=[[0, 1]], base=0, channel_multiplier=1)
            iota_col = cp.tile([P, 1], F32)
            nc.vector.tensor_copy(out=iota_col[:], in_=ic[:])
            ones_k = cp.tile([1, P], BF16)
            nc.gpsimd.memset(ones_k[:], 1.0)
            ones_row = cp.tile([1, 512], BF16)
            nc.gpsimd.memset(ones_row[:], 1.0)
            idf = cp.tile([P, P], F32)
            make_identity(nc, idf[:])
            idb = cp.tile([P, P], BF16)
            nc.vector.tensor_copy(out=idb[:], in_=idf[:])
            # weights
            ew1 = wp.tile([P, 5 * 256], BF16)
            nc.scalar.dma_start(out=ew1[:], in_=d_ew1[:])
            ew2 = wp.tile([P, 2 * 128], BF16)
            nc.scalar.dma_start(out=ew2[:], in_=d_ew2[:])
            nw1 = wp.tile([P, 3 * 256], BF16)
            nc.scalar.dma_start(out=nw1[:], in_=d_nw1[:])
            nw2 = wp.tile([P, 2 * 256], BF16)
            nc.scalar.dma_start(out=nw2[:], in_=d_nw2[:])
            eb1 = wp.tile([1, 256], BF16)
            nc.scalar.dma_start(out=eb1[:], in_=d_eb1[:])
            eb2 = wp.tile([1, 128], BF16)
            nc.scalar.dma_start(out=eb2[:], in_=d_eb2[:])
            nb1 = wp.tile([1, 256], BF16)
            nc.scalar.dma_start(out=nb1[:], in_=d_nb1[:])
            nb2 = wp.tile([1, 256], BF16)
            nc.scalar.dma_start(out=nb2[:], in_=d_nb2[:])

            nslab = (T + 3) // 4
            for g in range(G):
                # node window, transposed
                nwb = gp.tile([P, Dn], BF16, tag="nwb")
                nc.scalar.dma_start(out=nwb[:], in_=d_nbo[g * P:(g + 1) * P, :])
                nwT_ps = pmisc.tile([P, Dn], BF16, tag="misc")
                for c in range(2):
                    nc.tensor.transpose(
                        out=nwT_ps[:, c * P:(c + 1) * P],
                        in_=nwb[:, c * P:(c + 1) * P], identity=idb[:])
                nwT = gp.tile([P, Dn], BF16, tag="nwT_sb")
                nc.vector.tensor_copy(out=nwT[:], in_=nwT_ps[:])
                # Y = window @ W1_recv   [128n, 256h]
                y_ps = pmisc.tile([P, 256], F32, tag="misc")
                for c in range(2):
                    nc.tensor.matmul(
                        out=y_ps[:], lhsT=nwT[:, c * P:(c + 1) * P],
                        rhs=ew1[:, c * 256:(c + 1) * 256],
                        start=(c == 0), stop=(c == 1))
                ysb = gp.tile([P, 256], BF16, tag="ysb")
                nc.vector.tensor_copy(out=ysb[:], in_=y_ps[:])

                agg_ps = pagg.tile([P, P], F32, tag="agg")
                for s in range(nslab):
                    t0 = 4 * s
                    nt = min(4, T - t0)
                    W = nt * P
                    gt0 = g * T + t0
                    # index/rel loads
                    rc = sp.tile([P, 4], F32, tag="rc")
                    nc.scalar.dma_start(out=rc[:, :nt], in_=d_rc[:, gt0:gt0 + nt])
                    rr = sp.tile([1, 512], BF16, tag="rr")
                    nc.scalar.dma_start(out=rr[:, :W], in_=d_rr[:, gt0 * P:gt0 * P + W])
                    sn = sp.tile([P, 4], I32, tag="sn")
                    nc.scalar.dma_start(out=sn[:, :nt], in_=d_sn[:, gt0:gt0 + nt])
                    # gathers
                    snb = sp.tile([P, 4 * Dn], BF16, tag="snb")
                    esf = sp.tile([P, 4 * De], F32, tag="esf")
                    for j in range(nt):
                        nc.gpsimd.indirect_dma_start(
                            out=snb[:, j * Dn:(j + 1) * Dn], out_offset=None,
                            in_=d_nb[:],
                            in_offset=bass.IndirectOffsetOnAxis(ap=sn[:, j:j + 1], axis=0))
                        nc.scalar.dma_start(
                            out=esf[:, j * De:(j + 1) * De],
                            in_=d_es[(gt0 + j) * P:(gt0 + j + 1) * P, :])
                    # transposes: sndT (dma xbar), esT (PE)
                    sT0 = sp.tile([P, 512], BF16, tag="sT0")
                    sT1 = sp.tile([P, 512], BF16, tag="sT1")
                    for j in range(nt):
                        nc.sync.dma_start_transpose(
                            out=sT0[:, j * P:(j + 1) * P], in_=snb[:, j * Dn:j * Dn + P])
                        nc.sync.dma_start_transpose(
                            out=sT1[:, j * P:(j + 1) * P], in_=snb[:, j * Dn + P:(j + 1) * Dn])
                    esT_ps = psml.tile([P, 512], F32, tag="esT_ps")
                    for j in range(nt):
                        nc.tensor.transpose(
                            out=esT_ps[:, j * P:(j + 1) * P],
                            in_=esf[:, j * De:(j + 1) * De], identity=idf[:])
                    esT = sp.tile([P, 512], BF16, tag="esT")
                    nc.scalar.activation(out=esT[:, :W], in_=esT_ps[:, :W],
                                         func=mybir.ActivationFunctionType.Copy)
                    # one-hots
                    ohe = sp.tile([P, 4, P], BF16, tag="ohe")
                    nc.vector.tensor_tensor(
                        out=ohe[:, :nt, :],
                        in0=rc[:, :nt, None].to_broadcast([P, nt, P]),
                        in1=iota_row[:, None, :].to_broadcast([P, nt, P]),
                        op=mybir.AluOpType.is_equal)
                    rb_ps = pmisc.tile([P, 512], F32, tag="misc")
                    nc.tensor.matmul(out=rb_ps[:, :W], lhsT=ones_k[:], rhs=rr[:, :W],
                                     start=True, stop=True)
                    ohn = sp.tile([P, 512], BF16, tag="ohn")
                    nc.vector.tensor_tensor(
                        out=ohn[:, :W], in0=rb_ps[:, :W],
                        in1=iota_col[:].to_broadcast([P, W]),
                        op=mybir.AluOpType.is_equal)
                    # edge MLP layer 1 (feature-major h)
                    hts = []
                    for hc in range(2):
                        h_ps = ph1.tile([P, 512], F32, tag="h1t")
                        nc.tensor.matmul(out=h_ps[:, :W], lhsT=eb1[:, hc * P:(hc + 1) * P],
                                         rhs=ones_row[:, :W], start=True, stop=False)
                        nc.tensor.matmul(out=h_ps[:, :W], lhsT=ysb[:, hc * P:(hc + 1) * P],
                                         rhs=ohn[:, :W], start=False, stop=False)
                        for c in range(2):
                            nc.tensor.matmul(
                                out=h_ps[:, :W],
                                lhsT=ew1[:, (2 + c) * 256 + hc * P:(2 + c) * 256 + (hc + 1) * P],
                                rhs=(sT0 if c == 0 else sT1)[:, :W], start=False, stop=False)
                        nc.tensor.matmul(out=h_ps[:, :W],
                                         lhsT=ew1[:, 4 * 256 + hc * P:4 * 256 + (hc + 1) * P],
                                         rhs=esT[:, :W], start=False, stop=True)
                        ht = sp.tile([P, 512], BF16, tag=f"ht{hc}")
                        nc.scalar.activation(out=ht[:, :W], in_=h_ps[:, :W],
                                             func=mybir.ActivationFunctionType.Relu)
                        hts.append(ht)
                    # layer 2 -> enewT [De, e]
                    en_ps = psml.tile([P, 512], F32, tag="enT")
                    nc.tensor.matmul(out=en_ps[:, :W], lhsT=eb2[:], rhs=ones_row[:, :W],
                                     start=True, stop=False)
                    for c in range(2):
                        nc.tensor.matmul(out=en_ps[:, :W], lhsT=ew2[:, c * P:(c + 1) * P],
                                         rhs=hts[c][:, :W], start=False, stop=(c == 1))
                    enb = sp.tile([P, 512], BF16, tag="enb")
                    nc.scalar.activation(out=enb[:, :W], in_=en_ps[:, :W],
                                         func=mybir.ActivationFunctionType.Copy)
                    # transpose back to edge-major
                    em_ps = psml.tile([P, 512], BF16, tag="em")
                    for j in range(nt):
                        nc.tensor.transpose(
                            out=em_ps[:, j * P:(j + 1) * P],
                            in_=enb[:, j * P:(j + 1) * P], identity=idb[:])
                    emb = sp.tile([P, 512], BF16, tag="emb")
                    nc.vector.tensor_copy(out=emb[:, :W], in_=em_ps[:, :W])
                    # residual + scatter
                    eo = sp.tile([P, 512], F32, tag="eo")
                    nc.vector.tensor_add(out=eo[:, :W], in0=esf[:, :W], in1=em_ps[:, :W])
                    for j in range(nt):
                        nc.scalar.dma_start(
                            out=d_eo[(gt0 + j) * P:(gt0 + j + 1) * P, :],
                            in_=eo[:, j * De:(j + 1) * De])
                    # aggregation
                    for j in range(nt):
                        t = t0 + j
                        nc.tensor.matmul(
                            out=agg_ps[:], lhsT=emb[:, j * P:(j + 1) * P],
                            rhs=ohe[:, j, :], start=(t == 0), stop=(t == T - 1))
                # ---- node MLP for this group ----
                agg_bf = gp.tile([P, P], BF16, tag="aggbf")
                nc.vector.tensor_copy(out=agg_bf[:], in_=agg_ps[:])
                hn = []
                for hc in range(2):
                    hn_ps = pmisc.tile([P, P], F32, tag="misc")
                    nc.tensor.matmul(out=hn_ps[:], lhsT=nb1[:, hc * P:(hc + 1) * P],
                                     rhs=ones_row[:, :P], start=True, stop=False)
                    nc.tensor.matmul(out=hn_ps[:], lhsT=nw1[:, hc * P:(hc + 1) * P],
                                     rhs=agg_bf[:], start=False, stop=False)
                    for c in range(2):
                        nc.tensor.matmul(
                            out=hn_ps[:],
                            lhsT=nw1[:, (1 + c) * 256 + hc * P:(1 + c) * 256 + (hc + 1) * P],
                            rhs=nwT[:, c * P:(c + 1) * P], start=False, stop=(c == 1))
                    hnt = gp.tile([P, P], BF16, tag=f"hnt{hc}")
                    nc.scalar.activation(out=hnt[:], in_=hn_ps[:],
                                         func=mybir.ActivationFunctionType.Relu)
                    hn.append(hnt)
                nn_ps = pmisc.tile([P, 256], F32, tag="misc")
                for oc in range(2):
                    sl = slice(oc * P, (oc + 1) * P)
                    nc.tensor.matmul(out=nn_ps[:, sl], lhsT=nb2[:, sl],
                                     rhs=ones_row[:, :P], start=True, stop=False)
                    for c in range(2):
                        nc.tensor.matmul(
                            out=nn_ps[:, sl],
                            lhsT=nw2[:, c * 256 + oc * P:c * 256 + (oc + 1) * P],
                            rhs=hn[c][:], start=False, stop=(c == 1))
                nrT = gp.tile([P, 256], F32, tag="nrT")
                for oc in range(2):
                    nc.scalar.dma_start(
                        out=nrT[:, oc * P:(oc + 1) * P],
                        in_=d_nto[oc * P:(oc + 1) * P, g * P:(g + 1) * P])
                noT = gp.tile([P, 256], F32, tag="noT")
                nc.vector.tensor_add(out=noT[:], in0=nrT[:], in1=nn_ps[:])
                for oc in range(2):
                    nc.scalar.dma_start(
                        out=d_no[oc * P:(oc + 1) * P, g * P:(g + 1) * P],
                        in_=noT[:, oc * P:(oc + 1) * P])
    nc.finalize()
    _cache[T] = nc
    return nc


def _prep_weights(eW1, eW2, nW1, nW2):
    def lay(w, kc, m):
        return np.ascontiguousarray(
            w.reshape(kc, P, m).transpose(1, 0, 2).reshape(P, kc * m)).astype(bf)
    return dict(
        ew1=lay(np.asarray(eW1, np.float32), 5, 256),
        ew2=lay(np.asarray(eW2, np.float32), 2, 128),
        nw1=lay(np.asarray(nW1, np.float32), 3, 256),
        nw2=lay(np.asarray(nW2, np.float32), 2, 256),
    )


def kernel(nodes, edges, senders, receivers, eW1, eb1, eW2, eb2, nW1, nb1, nW2, nb2):
    nodes = np.asarray(nodes, np.float32)
    edges = np.asarray(edges, np.float32)
    senders = np.asarray(senders, np.int32)
    receivers = np.asarray(receivers, np.int32)

    wmaps = _prep_weights(eW1, eW2, nW1, nW2)
    bmaps = dict(
        eb1=np.asarray(eb1, np.float32).reshape(1, 256).astype(bf),
        eb2=np.asarray(eb2, np.float32).reshape(1, 128).astype(bf),
        nb1=np.asarray(nb1, np.float32).reshape(1, 256).astype(bf),
        nb2=np.asarray(nb2, np.float32).reshape(1, 256).astype(bf),
    )

    # host: sort edges by receiver, split into per-core / per-group segments
    perms, starts_all, counts_all = [], [], []
    Tmax = 1
    for b in range(B):
        perm = np.argsort(receivers[b], kind="stable")
        rs = receivers[b][perm]
        counts = np.bincount(rs >> 7, minlength=N // P)
        starts = np.concatenate([[0], np.cumsum(counts)])
        perms.append((perm, rs))
        starts_all.append(starts)
        counts_all.append(counts)
        Tmax = max(Tmax, int(np.ceil(counts.max() / P)))
    T = int(Tmax)
    TT = G * T
    Ep = TT * P

    in_maps = []
    core_meta = []
    for b in range(B):
        perm, rs = perms[b]
        starts = starts_all[b]
        counts = counts_all[b]
        nb16 = nodes[b].astype(bf)
        for q in range(Q):
            sg = np.zeros(Ep, np.int32)
            sn = np.zeros(Ep, np.int32)
            rc = np.full(Ep, 200.0, np.float32)
            valid_idx, valid_pos = [], []
            for gl in range(G):
                gg = q * G + gl
                c = int(counts[gg])
                seg = perm[starts[gg]:starts[gg] + c]
                base = gl * T * P
                sg[base:base + c] = seg
                sn[base:base + c] = senders[b][seg]
                rc[base:base + c] = (rs[starts[gg]:starts[gg] + c] - gg * P).astype(np.float32)
                valid_idx.append(seg)
                valid_pos.append(np.arange(base, base + c))
            tilemaj = lambda a: np.ascontiguousarray(a.reshape(TT, P).T)
            m = dict(
                nodes_bf=nb16,
                nodes_bf_own=nb16[q * NPC:(q + 1) * NPC],
                nodesT_own=np.ascontiguousarray(nodes[b, q * NPC:(q + 1) * NPC].T),
                edges_s=edges[b][sg],
                g_snd=tilemaj(sn),
                g_rc=tilemaj(rc).astype(np.float32),
                g_rr=rc.reshape(1, Ep).astype(bf),
                **wmaps, **bmaps,
            )
            in_maps.append(m)
            core_meta.append((b, q, np.concatenate(valid_idx), np.concatenate(valid_pos)))

    nc = _build(T)
    global _last_in_maps, _last_T
    _last_in_maps, _last_T = in_maps, T
    res = run_bass_kernel_spmd(nc, in_maps, list(range(8)))

    nodes_out = np.empty((B, N, Dn), np.float32)
    edges_out = np.empty((B, E, De), np.float32)
    for ci, (bq, q, vidx, vpos) in enumerate(core_meta):
        r = res.results[ci]
        nodes_out[bq, q * NPC:(q + 1) * NPC] = r["noutT"].T
        edges_out[bq][vidx] = r["eout"][vpos]
    return nodes_out, edges_out


# revision 11
# speedup vs baseline: 1.4015x; 1.4015x over previous
"""InteractionNetworkLayer on 8 TRN2 cores.

Sharding: core = (b, q); b = batch (2), q = node-quarter (4). Each core owns
nodes [q*4096,(q+1)*4096) of batch b, and (via host argsort by receiver) the
contiguous run of edges whose receiver falls in that range. Edges are grouped
by 128-node windows (32 groups/core), each group padded to T tiles of 128
edges (T = global max, data-dependent).

Device dataflow (feature-major activations, bf16 matmuls):
  - send feats: indirect-DMA gather of bf16 node rows + DMA-transpose
  - recv feats: never gathered; folded through Y = nodes_window @ W1_recv and
    a one-hot matmul (receivers sorted => window-local)
  - edge MLP: h1T[h,e] accumulated in PSUM from (bias, Y@onehot, W1@sndT,
    W1@esT); relu on ACT; enewT = eW2@hT
  - aggregation: aggT[De,n] += enew_em.T-matmul with edge-major one-hots
  - node MLP: feature-major end-to-end, residual via host-transposed nodes
"""
import numpy as np
import ml_dtypes
from concourse import bacc, mybir
import concourse.bass as bass
from concourse.tile import TileContext
from concourse.masks import make_identity
from concourse.bass_utils import run_bass_kernel_spmd

BF16 = mybir.dt.bfloat16
F32 = mybir.dt.float32
I32 = mybir.dt.int32
P = 128
B, N, E, Dn, De, H = 2, 16384, 131072, 256, 128, 256
Q = 4              # cores per batch
NPC = N // Q       # 4096 nodes per core
G = NPC // P       # 32 groups per core
bf = ml_dtypes.bfloat16

_cache = {}


def _build(T):
    if T in _cache:
        return _cache[T]
    TT = G * T
    Ep = TT * P
    nc = bacc.Bacc(None, target_bir_lowering=False)

    d_nb = nc.declare_dram_parameter("nodes_bf", [N, Dn], BF16, isOutput=False)
    d_nbo = nc.declare_dram_parameter("nodes_bf_own", [NPC, Dn], BF16, isOutput=False)
    d_nto = nc.declare_dram_parameter("nodesT_own", [Dn, NPC], F32, isOutput=False)
    d_es = nc.declare_dram_parameter("edges_s", [Ep, De], F32, isOutput=False)
    d_ew1 = nc.declare_dram_parameter("ew1", [P, 5 * 256], BF16, isOutput=False)
    d_ew2 = nc.declare_dram_parameter("ew2", [P, 2 * 128], BF16, isOutput=False)
    d_nw1 = nc.declare_dram_parameter("nw1", [P, 3 * 256], BF16, isOutput=False)
    d_nw2 = nc.declare_dram_parameter("nw2", [P, 2 * 256], BF16, isOutput=False)
    d_eb1 = nc.declare_dram_parameter("eb1", [1, 256], BF16, isOutput=False)
    d_eb2 = nc.declare_dram_parameter("eb2", [1, 128], BF16, isOutput=False)
    d_nb1 = nc.declare_dram_parameter("nb1", [1, 256], BF16, isOutput=False)
    d_nb2 = nc.declare_dram_parameter("nb2", [1, 256], BF16, isOutput=False)
    d_sn = nc.declare_dram_parameter("g_snd", [P, TT], I32, isOutput=False)
    d_rc = nc.declare_dram_parameter("g_rc", [P, TT], F32, isOutput=False)
    d_rr = nc.declare_dram_parameter("g_rr", [1, Ep], BF16, isOutput=False)
    d_eo = nc.declare_dram_parameter("eout", [Ep, De], F32, isOutput=True)
    d_no = nc.declare_dram_parameter("noutT", [Dn, NPC], F32, isOutput=True)

    with TileContext(nc) as tc:
        with (
            tc.tile_pool(name="const", bufs=1) as cp,
            tc.tile_pool(name="wts", bufs=1) as wp,
            tc.tile_pool(name="work", bufs=4) as sp,
            tc.tile_pool(name="grp", bufs=3) as gp,
            tc.tile_pool(name="ph1", bufs=3, space="PSUM") as ph1,
            tc.tile_pool(name="psml", bufs=1, space="PSUM") as psml,
            tc.tile_pool(name="pagg", bufs=1, space="PSUM") as pagg,
            tc.tile_pool(name="pmisc", bufs=1, space="PSUM") as pmisc,
        ):
            # constants
            ii = cp.tile([P, P], I32)
            nc.gpsimd.iota(ii[:], pattern=[[1, P]], base=0, channel_multiplier=0)
            iota_row = cp.tile([P, P], F32)
            nc.vector.tensor_copy(out=iota_row[:], in_=ii[:])
            ic = cp.tile([P, 1], I32)
            nc.gpsimd.iota(ic[:], pattern=[[0, 1]], base=0, channel_multiplier=1)
            iota_col = cp.tile([P, 1], F32)
            nc.vector.tensor_copy(out=iota_col[:], in_=ic[:])
            ones_k = cp.tile([1, P], BF16)
            nc.gpsimd.memset(ones_k[:], 1.0)
            ones_row = cp.tile([1, 512], BF16)
            nc.gpsimd.memset(ones_row[:], 1.0)
            idf = cp.tile([P, P], F32)
            make_identity(nc, idf[:])
            idb = cp.tile([P, P], BF16)
            nc.vector.tensor_copy(out=idb[:], in_=idf[:])
            # weights
            ew1 = wp.tile([P, 5 * 256], BF16)
            nc.scalar.dma_start(out=ew1[:], in_=d_ew1[:])
            ew2 = wp.tile([P, 2 * 128], BF16)
            nc.scalar.dma_start(out=ew2[:], in_=d_ew2[:])
            nw1 = wp.tile([P, 3 * 256], BF16)
            nc.scalar.dma_start(out=nw1[:], in_=d_nw1[:])
            nw2 = wp.tile([P, 2 * 256], BF16)
            nc.scalar.dma_start(out=nw2[:], in_=d_nw2[:])
            eb1 = wp.tile([1, 256], BF16)
            nc.scalar.dma_start(out=eb1[:], in_=d_eb1[:])
            eb2 = wp.tile([1, 128], BF16)
            nc.scalar.dma_start(out=eb2[:], in_=d_eb2[:])
            nb1 = wp.tile([1, 256], BF16)
            nc.scalar.dma_start(out=nb1[:], in_=d_nb1[:])
            nb2 = wp.tile([1, 256], BF16)
            nc.scalar.dma_start(out=nb2[:], in_=d_nb2[:])

            nslab = (T + 3) // 4
            for g in range(G):
                # node window, transposed
                nwb = gp.tile([P, Dn], BF16, tag="nwb")
                nc.scalar.dma_start(out=nwb[:], in_=d_nbo[g * P:(g + 1) * P, :])
                nwT_ps = pmisc.tile([P, Dn], BF16, tag="misc")
                for c in range(2):
                    nc.tensor.transpose(
                        out=nwT_ps[:, c * P:(c + 1) * P],
                        in_=nwb[:, c * P:(c + 1) * P], identity=idb[:])
                nwT = gp.tile([P, Dn], BF16, tag="nwT_sb")
                nc.vector.tensor_copy(out=nwT[:], in_=nwT_ps[:])
                # Y = window @ W1_recv   [128n, 256h]
                y_ps = pmisc.tile([P, 256], F32, tag="misc")
                for c in range(2):
                    nc.tensor.matmul(
                        out=y_ps[:], lhsT=nwT[:, c * P:(c + 1) * P],
                        rhs=ew1[:, c * 256:(c + 1) * 256],
                        start=(c == 0), stop=(c == 1))
                ysb = gp.tile([P, 256], BF16, tag="ysb")
                nc.vector.tensor_copy(out=ysb[:], in_=y_ps[:])

                agg_ps = pagg.tile([P, P], F32, tag="agg")
                for s in range(nslab):
                    t0 = 4 * s
                    nt = min(4, T - t0)
                    W = nt * P
                    gt0 = g * T + t0
                    # index/rel loads
                    rc = sp.tile([P, 4], F32, tag="rc")
                    nc.scalar.dma_start(out=rc[:, :nt], in_=d_rc[:, gt0:gt0 + nt])
                    rr = sp.tile([1, 512], BF16, tag="rr")
                    nc.scalar.dma_start(out=rr[:, :W], in_=d_rr[:, gt0 * P:gt0 * P + W])
                    sn = sp.tile([P, 4], I32, tag="sn")
                    nc.scalar.dma_start(out=sn[:, :nt], in_=d_sn[:, gt0:gt0 + nt])
                    # gathers
                    snb = sp.tile([P, 4 * Dn], BF16, tag="snb")
                    esf = sp.tile([P, 4 * De], F32, tag="esf")
                    for j in range(nt):
                        nc.gpsimd.indirect_dma_start(
                            out=snb[:, j * Dn:(j + 1) * Dn], out_offset=None,
                            in_=d_nb[:],
                            in_offset=bass.IndirectOffsetOnAxis(ap=sn[:, j:j + 1], axis=0))
                        nc.scalar.dma_start(
                            out=esf[:, j * De:(j + 1) * De],
                            in_=d_es[(gt0 + j) * P:(gt0 + j + 1) * P, :])
                    # transposes: sndT (dma xbar), esT (PE)
                    sT0 = sp.tile([P, 512], BF16, tag="sT0")
                    sT1 = sp.tile([P, 512], BF16, tag="sT1")
                    for j in range(nt):
                        nc.sync.dma_start_transpose(
                            out=sT0[:, j * P:(j + 1) * P], in_=snb[:, j * Dn:j * Dn + P])
                        nc.sync.dma_start_transpose(
                            out=sT1[:, j * P:(j + 1) * P], in_=snb[:, j * Dn + P:(j + 1) * Dn])
                    esT_ps = psml.tile([P, 512], F32, tag="esT_ps")
                    for j in range(nt):
                        nc.tensor.transpose(
                            out=esT_ps[:, j * P:(j + 1) * P],
                            in_=esf[:, j * De:(j + 1) * De], identity=idf[:])
                    esT = sp.tile([P, 512], BF16, tag="esT")
                    nc.scalar.activation(out=esT[:, :W], in_=esT_ps[:, :W],
                                         func=mybir.ActivationFunctionType.Copy)
                    # one-hots
                    ohe = sp.tile([P, 4, P], BF16, tag="ohe")
                    nc.vector.tensor_tensor(
                        out=ohe[:, :nt, :],
                        in0=rc[:, :nt, None].to_broadcast([P, nt, P]),
                        in1=iota_row[:, None, :].to_broadcast([P, nt, P]),
                        op=mybir.AluOpType.is_equal)
                    rb_ps = pmisc.tile([P, 512], F32, tag="misc")
                    nc.tensor.matmul(out=rb_ps[:, :W], lhsT=ones_k[:], rhs=rr[:, :W],
                                     start=True, stop=True)
                    ohn = sp.tile([P, 512], BF16, tag="ohn")
                    nc.vector.tensor_tensor(
                        out=ohn[:, :W], in0=rb_ps[:, :W],
                        in1=iota_col[:].to_broadcast([P, W]),
                        op=mybir.AluOpType.is_equal)
                    # edge MLP layer 1 (feature-major h)
                    hts = []
                    for hc in range(2):
                        h_ps = ph1.tile([P, 512], F32, tag="h1t")
                        nc.tensor.matmul(out=h_ps[:, :W], lhsT=eb1[:, hc * P:(hc + 1) * P],
                                         rhs=ones_row[:, :W], start=True, stop=False)
                        nc.tensor.matmul(out=h_ps[:, :W], lhsT=ysb[:, hc * P:(hc + 1) * P],
                                         rhs=ohn[:, :W], start=False, stop=False)
                        for c in range(2):
                            nc.tensor.matmul(
                                out=h_ps[:, :W],
                                lhsT=ew1[:, (2 + c) * 256 + hc * P:(2 + c) * 256 + (hc + 1) * P],
                                rhs=(sT0 if c == 0 else sT1)[:, :W], start=False, stop=False)
                        nc.tensor.matmul(out=h_ps[:, :W],
                                         lhsT=ew1[:, 4 * 256 + hc * P:4 * 256 + (hc + 1) * P],
                                         rhs=esT[:, :W], start=False, stop=True)
                        ht = sp.tile([P, 512], BF16, tag=f"ht{hc}")
                        nc.scalar.activation(out=ht[:, :W], in_=h_ps[:, :W],
                                             func=mybir.ActivationFunctionType.Relu)
                        hts.append(ht)
                    # layer 2 -> enewT [De, e]
                    en_ps = psml.tile([P, 512], F32, tag="enT")
                    nc.tensor.matmul(out=en_ps[:, :W], lhsT=eb2[:], rhs=ones_row[:, :W],
                                     start=True, stop=False)
                    for c in range(2):
                        nc.tensor.matmul(out=en_ps[:, :W], lhsT=ew2[:, c * P:(c + 1) * P],
                                         rhs=hts[c][:, :W], start=False, stop=(c == 1))
                    enb = sp.tile([P, 512], BF16, tag="enb")
                    nc.scalar.activation(out=enb[:, :W], in_=en_ps[:, :W],
                                         func=mybir.ActivationFunctionType.Copy)
                    # transpose back to edge-major
                    em_ps = psml.tile([P, 512], BF16, tag="em")
                    for j in range(nt):
                        nc.tensor.transpose(
                            out=em_ps[:, j * P:(j + 1) * P],
                            in_=enb[:, j * P:(j + 1) * P], identity=idb[:])
                    emb = sp.tile([P, 512], BF16, tag="emb")
                    nc.vector.tensor_copy(out=emb[:, :W], in_=em_ps[:, :W])
                    # residual + scatter
                    eo = sp.tile([P, 512], F32, tag="eo")
                    nc.vector.tensor_add(out=eo[:, :W], in0=esf[:, :W], in1=em_ps[:, :W])
                    for j in range(nt):
                        nc.scalar.dma_start(
                            out=d_eo[(gt0 + j) * P:(gt0 + j + 1) * P, :],
                            in_=eo[:, j * De:(j + 1) * De])
                    # aggregation
                    for j in range(nt):
                        t = t0 + j
                        nc.tensor.matmul(
                            out=agg_ps[:], lhsT=emb[:, j * P:(j + 1) * P],
                            rhs=ohe[:, j, :], start=(t == 0), stop=(t == T - 1))
                # ---- node MLP for this group ----
                agg_bf = gp.tile([P, P], BF16, tag="aggbf")
                nc.vector.tensor_copy(out=agg_bf[:], in_=agg_ps[:])
                hn = []
                for hc in range(2):
                    hn_ps = pmisc.tile([P, P], F32, tag="misc")
                    nc.tensor.matmul(out=hn_ps[:], lhsT=nb1[:, hc * P:(hc + 1) * P],
                                     rhs=ones_row[:, :P], start=True, stop=False)
                    nc.tensor.matmul(out=hn_ps[:], lhsT=nw1[:, hc * P:(hc + 1) * P],
                                     rhs=agg_bf[:], start=False, stop=False)
                    for c in range(2):
                        nc.tensor.matmul(
                            out=hn_ps[:],
                            lhsT=nw1[:, (1 + c) * 256 + hc * P:(1 + c) * 256 + (hc + 1) * P],
                            rhs=nwT[:, c * P:(c + 1) * P], start=False, stop=(c == 1))
                    hnt = gp.tile([P, P], BF16, tag=f"hnt{hc}")
                    nc.scalar.activation(out=hnt[:], in_=hn_ps[:],
                                         func=mybir.ActivationFunctionType.Relu)
                    hn.append(hnt)
                nn_ps = pmisc.tile([P, 256], F32, tag="misc")
                for oc in range(2):
                    sl = slice(oc * P, (oc + 1) * P)
                    nc.tensor.matmul(out=nn_ps[:, sl], lhsT=nb2[:, sl],
                                     rhs=ones_row[:, :P], start=True, stop=False)
                    for c in range(2):
                        nc.tensor.matmul(
                            out=nn_ps[:, sl],
                            lhsT=nw2[:, c * 256 + oc * P:c * 256 + (oc + 1) * P],
                            rhs=hn[c][:], start=False, stop=(c == 1))
                nrT = gp.tile([P, 256], F32, tag="nrT")
                for oc in range(2):
                    nc.scalar.dma_start(
                        out=nrT[:, oc * P:(oc + 1) * P],
                        in_=d_nto[oc * P:(oc + 1) * P, g * P:(g + 1) * P])
                noT = gp.tile([P, 256], F32, tag="noT")
                nc.vector.tensor_add(out=noT[:], in0=nrT[:], in1=nn_ps[:])
                for oc in range(2):
                    nc.scalar.dma_start(
                        out=d_no[oc * P:(oc + 1) * P, g * P:(g + 1) * P],
                        in_=noT[:, oc * P:(oc + 1) * P])
    nc.finalize()
    _cache[T] = nc
    return nc


def _prep_weights(eW1, eW2, nW1, nW2):
    def lay(w, kc, m):
        return np.ascontiguousarray(
            w.reshape(kc, P, m).transpose(1, 0, 2).reshape(P, kc * m)).astype(bf)
    return dict(
        ew1=lay(np.asarray(eW1, np.float32), 5, 256),
        ew2=lay(np.asarray(eW2, np.float32), 2, 128),
        nw1=lay(np.asarray(nW1, np.float32), 3, 256),
        nw2=lay(np.asarray(nW2, np.float32), 2, 256),
    )


def kernel(nodes, edges, senders, receivers, eW1, eb1, eW2, eb2, nW1, nb1, nW2, nb2):
    nodes = np.asarray(nodes, np.float32)
    edges = np.asarray(edges, np.float32)
    senders = np.asarray(senders, np.int32)
    receivers = np.asarray(receivers, np.int32)

    wmaps = _prep_weights(eW1, eW2, nW1, nW2)
    bmaps = dict(
        eb1=np.asarray(eb1, np.float32).reshape(1, 256).astype(bf),
        eb2=np.asarray(eb2, np.float32).reshape(1, 128).astype(bf),
        nb1=np.asarray(nb1, np.float32).reshape(1, 256).astype(bf),
        nb2=np.asarray(nb2, np.float32).reshape(1, 256).astype(bf),
    )

    # host: sort edges by receiver, split into per-core / per-group segments
    perms, starts_all, counts_all = [], [], []
    Tmax = 1
    for b in range(B):
        perm = np.argsort(receivers[b], kind="stable")
        rs = receivers[b][perm]
        counts = np.bincount(rs >> 7, minlength=N // P)
        starts = np.concatenate([[0], np.cumsum(counts)])
        perms.append((perm, rs))
        starts_all.append(starts)
        counts_all.append(counts)
        Tmax = max(Tmax, int(np.ceil(counts.max() / P)))
    T = int(Tmax)
    TT = G * T
    Ep = TT * P

    in_maps = []
    core_meta = []
    for b in range(B):
        perm, rs = perms[b]
        starts = starts_all[b]
        counts = counts_all[b]
        nb16 = nodes[b].astype(bf)
        for q in range(Q):
            sg = np.zeros(Ep, np.int32)
            sn = np.zeros(Ep, np.int32)
            rc = np.full(Ep, 200.0, np.float32)
            valid_idx, valid_pos = [], []
            for gl in range(G):
                gg = q * G + gl
                c = int(counts[gg])
                seg = perm[starts[gg]:starts[gg] + c]
                base = gl * T * P
                sg[base:base + c] = seg
                sn[base:base + c] = senders[b][seg]
                rc[base:base + c] = (rs[starts[gg]:starts[gg] + c] - gg * P).astype(np.float32)
                valid_idx.append(seg)
                valid_pos.append(np.arange(base, base + c))
            tilemaj = lambda a: np.ascontiguousarray(a.reshape(TT, P).T)
            m = dict(
                nodes_bf=nb16,
                nodes_bf_own=nb16[q * NPC:(q + 1) * NPC],
                nodesT_own=np.ascontiguousarray(nodes[b, q * NPC:(q + 1) * NPC].T),
                edges_s=edges[b][sg],
                g_snd=tilemaj(sn),
                g_rc=tilemaj(rc).astype(np.float32),
                g_rr=rc.reshape(1, Ep).astype(bf),
                **wmaps, **bmaps,
            )
            in_maps.append(m)
            core_meta.append((b, q, np.concatenate(valid_idx), np.concatenate(valid_pos)))

    nc = _build(T)
    global _last_in_maps, _last_T
    _last_in_maps, _last_T = in_maps, T
    res = run_bass_kernel_spmd(nc, in_maps, list(range(8)))

    nodes_out = np.empty((B, N, Dn), np.float32)
    edges_out = np.empty((B, E, De), np.float32)
    for ci, (bq, q, vidx, vpos) in enumerate(core_meta):
        r = res.results[ci]
        nodes_out[bq, q * NPC:(q + 1) * NPC] = r["noutT"].T
        edges_out[bq][vidx] = r["eout"][vpos]
    return nodes_out, edges_out
